# revision 56
# baseline (speedup 1.0000x reference)
"""Trainium2 Bass kernel for a dense transformer block (pre-LN, masked attention).

Sharding: data-parallel over batch B=8 across the 8 NeuronCores — each core
processes one full batch element [T=1024, C=1024]; weights are replicated.
No collectives needed.

Per-core dataflow (single NeuronCore):
  - x loaded token-major [128, 8, 1024] (tokens on partitions).
  - LN1 stats token-major (bn_stats/bn_aggr), normalize on ScalarE,
    PE-transpose to feature-major xnT [C, T] stored fp8(e4m3).
  - QKV: fp8 DoubleRow matmuls (2 contraction rows per PE cell, ~1.5x) with
    weights pre-scaled by 64 on host so they sit in e4m3 normal range; the
    1/64 is folded into the PSUM eviction scale.  Q/K stored bf16 (the QK
    matmul has K=64 contraction, which fp8 cannot speed up).  V stored fp8
    per head pair as [even64 | ones | ones | odd64] (130 cols): the two ones
    columns make both heads' AV matmuls also emit the softmax row-sums.
  - QK: bf16, keys on psum partitions.  The two heads of a pair live on
    partition halves of q_sb/k_sb, and their matmuls are issued back-to-back
    so the PE runs them concurrently in disjoint row groups (K=64 each).
  - softmax: exp on ScalarE with the -30000 key-padding mask as bias
    (masked keys give exp == 0 exactly); st stored fp8 unnormalized.
  - AV: fp8 DoubleRow over key-tile pairs; even head -> psum rows 0:65
    (row 64 = sums), odd head -> rows 63:128 (row 63 = sums).  Eviction
    multiplies by broadcast 64/rowsum and casts yT to fp8.
  - proj: fp8 DoubleRow, eviction fused with residual add (scale 1/4096).
  - LN2 -> bf16 xn2T, MLP in bf16 (fp8 would breach the error budget):
    relu fc1 feature-major, fc2 token-major + residual, DMA out.
  - MLP: W_fc2 is streamed exactly once per T-half (both column halves per
    load) with 8 PSUM accumulators in 4 paired banks, kk blocked by 4 so each
    accumulator sees back-to-back matmuls (keeps the PE HAM clock warm).
    Weights are host-pre-tiled so every slab DMA is contiguous per partition.
"""

import os
import sys
import numpy as np
import ml_dtypes

for _p in ("/opt/trn_rl_repo", "/opt/pypackages"):
    if os.path.isdir(_p) and _p not in sys.path:
        sys.path.append(_p)

import concourse.bass as bass
import concourse.mybir as mybir
import concourse.tile as tile
from concourse import bacc
from concourse.bass_utils import run_bass_kernel_spmd

P = 128
B, T, C = 8, 1024, 1024
NH, HD = 16, 64
FF = 4 * C
EPS = 1e-5
NT = T // P      # 8 token tiles
NCD = C // P     # 8 feature tiles
NFF = FF // P    # 32 ff tiles
N_CORES = 8
MASK_VAL = -40.0
WS = 64.0        # fp8 weight pre-scale
INV_WS = 1.0 / WS
H2C = 0.5        # h2 centering offset for fp8 FC2 (zeros stay exact)
KK8 = 8          # leading FC1 kk-tiles in fp8 (rest bf16)
# Schraudolph fast-exp constants (DVE magic-number path), /256 scaled so the
# +2^23 magic add stays in range; the <<8 shift restores the exponent field.
FE_A = float(2**23 / np.log(2)) / 256.0
FE_B = ((127 << 23) - 486411) / 256.0 + float(2**23)

F32 = mybir.dt.float32
I32 = mybir.dt.int32
BF16 = mybir.dt.bfloat16
F8 = mybir.dt.float8e4
AF = mybir.ActivationFunctionType
OP = mybir.AluOpType
DR = mybir.MatmulPerfMode.DoubleRow

bf16 = ml_dtypes.bfloat16
f8 = ml_dtypes.float8_e4m3


# --------------------------------------------------------------------------
# host-side preparation: fold LN gains/biases into weights, build mask rows
# --------------------------------------------------------------------------
def _host_prep(x, seq_ls, ln1_g, ln1_b, w_qkv, b_qkv, w_proj, b_proj,
               ln2_g, ln2_b, w_fc, b_fc, w_fc2, b_fc2):
    f32 = np.float32
    ln1_g, ln1_b = ln1_g.astype(f32), ln1_b.astype(f32)
    w_qkv = w_qkv.astype(f32)

    wqkv_eff = ln1_g[:, None] * w_qkv                     # [C, 3C]
    bqkv_eff = ln1_b @ w_qkv + b_qkv.astype(f32)          # [3C]
    scale = np.float32(1.0 / np.sqrt(HD))
    wq = wqkv_eff[:, :C] * scale
    bq = bqkv_eff[:C] * scale
    wk = wqkv_eff[:, C:2 * C]
    bk = bqkv_eff[C:2 * C]
    wv = wqkv_eff[:, 2 * C:]
    bv = bqkv_eff[2 * C:]

    bproj_eff = bv @ w_proj.astype(f32) + b_proj.astype(f32)   # [C]

    wfc_eff = ln2_g.astype(f32)[:, None] * w_fc.astype(f32)    # [C, FF]
    bfc_eff = ln2_b.astype(f32) @ w_fc.astype(f32) + b_fc.astype(f32)

    wqk = np.concatenate([wq, wk], axis=1)                # [C, 2C]
    bqk_t = np.concatenate([bq, bk]).reshape(16, P).T.copy()   # [P, 16]
    bfc_t = bfc_eff.reshape(NFF, P).T.copy()              # [P, 32]

    def to_f8(w):
        return np.clip(w * WS, -240.0, 240.0).astype(f8)

    def tile_km(w, mw):
        # [K, M] -> [M//mw, P, K//P, mw] so a [P, K//P, mw] slab is contiguous
        K_, M_ = w.shape
        return np.ascontiguousarray(
            w.reshape(K_ // P, P, M_ // mw, mw).transpose(2, 1, 0, 3)
        ).reshape(M_ // mw, P, (K_ // P) * mw)

    shared = {
        "wqk": tile_km(to_f8(wqk), P),              # [16, P, 8*128] fp8
        "wv": tile_km(to_f8(wv), 512),              # [2, P, 8*512] fp8
        "bqk_t": bqk_t.astype(f32),
        "wproj": tile_km(to_f8(w_proj.astype(f32)), 512),  # [2, P, 8*512] fp8
        "bprojrow": bproj_eff.reshape(1, C).astype(bf16),
        "wfc8": tile_km(to_f8(wfc_eff[:, :KK8 * P]), P),   # [8, P, 8*128] fp8
        "wfc": tile_km(wfc_eff[:, KK8 * P:].astype(bf16), P),  # [24, P, 8*128]
        "bfc_t": bfc_t.astype(f32),
        # FC2 fp8: kk-pair-tiled rhs [ch, p, j, e, col] for DoubleRow; h2 is
        # centered by H2C on device, compensated via the bias (rank-1 fold).
        "wfc2": np.ascontiguousarray(
            to_f8(w_fc2.astype(f32)).reshape(16, 2, P, 2, 512)
            .transpose(3, 2, 0, 1, 4)).reshape(2, P, 16 * 1024),
        "bfc2row": (b_fc2.astype(f32) + H2C *
                    to_f8(w_fc2.astype(f32)).astype(f32).sum(0) / WS)
                   .reshape(1, C).astype(bf16),
    }
    per_core = []
    t_idx = np.arange(T)
    for b in range(B):
        mask = np.where(t_idx < int(seq_ls[b]), 0.0, MASK_VAL).astype(f32)
        per_core.append({
            "x": np.ascontiguousarray(x[b]).astype(f32),
            "mask_cols": mask.reshape(NT, P).T.copy(),   # [P, NT]
            "mask_ab": (FE_B + mask * FE_A).astype(f32)
                       .reshape(NT, P).T.copy(),          # [P, NT]
        })
    return shared, per_core


# --------------------------------------------------------------------------
# kernel build (single NeuronCore program, SPMD across 8 cores)
# --------------------------------------------------------------------------
def _build_nc(phases=99, repeat=1):
    nc = bacc.Bacc("TRN2", target_bir_lowering=False, debug=False,
                   num_devices=N_CORES)

    x_d = nc.dram_tensor("x", [T, C], F32, kind="ExternalInput").ap()
    mask_cols_d = nc.dram_tensor("mask_cols", [P, NT], F32,
                                 kind="ExternalInput").ap()
    wqk_d = nc.dram_tensor("wqk", [16, P, 8 * P], F8,
                           kind="ExternalInput").ap()
    wv_d = nc.dram_tensor("wv", [2, P, 8 * 512], F8,
                          kind="ExternalInput").ap()
    bqk_t_d = nc.dram_tensor("bqk_t", [P, 16], F32, kind="ExternalInput").ap()
    wproj_d = nc.dram_tensor("wproj", [2, P, 8 * 512], F8,
                             kind="ExternalInput").ap()
    bprojrow_d = nc.dram_tensor("bprojrow", [1, C], BF16, kind="ExternalInput").ap()
    wfc8_d = nc.dram_tensor("wfc8", [KK8, P, 8 * P], F8,
                            kind="ExternalInput").ap()
    wfc_d = nc.dram_tensor("wfc", [NFF - KK8, P, 8 * P], BF16,
                           kind="ExternalInput").ap()
    mask_ab_d = nc.dram_tensor("mask_ab", [P, NT], F32,
                               kind="ExternalInput").ap()
    bfc_t_d = nc.dram_tensor("bfc_t", [P, NFF], F32, kind="ExternalInput").ap()
    wfc2_d = nc.dram_tensor("wfc2", [2, P, 16 * 1024], F8,
                            kind="ExternalInput").ap()
    bfc2row_d = nc.dram_tensor("bfc2row", [1, C], BF16, kind="ExternalInput").ap()
    out_d = nc.dram_tensor("out", [T, C], F32, kind="ExternalOutput").ap()

    # DRAM access-pattern views
    x_v = x_d.rearrange("(i p) c -> p i c", p=P)          # [P, NT, C]
    out_v = out_d.rearrange("(i p) c -> p i c", p=P)
    wqk_v = wqk_d.rearrange("m p (k c) -> m p k c", k=NCD)    # [16,P,8,128]
    wv_v = wv_d.rearrange("m p (k c) -> m p k c", k=NCD)       # [2,P,8,512]
    wproj_v = wproj_d.rearrange("m p (k c) -> m p k c", k=NCD)
    wfc8_v = wfc8_d.rearrange("m p (k c) -> m p k c", k=NCD)   # [8,P,8,128]
    wfc_v = wfc_d.rearrange("m p (k c) -> m p k c", k=NCD)     # [24,P,8,128]

    with tile.TileContext(nc) as tc:
        with (
            tc.tile_pool(name="persist", bufs=1) as pp,
            tc.tile_pool(name="qpool", bufs=2) as qpool,
            tc.tile_pool(name="kpool", bufs=2) as kpool,
            tc.tile_pool(name="stpool", bufs=2) as stpool,
            tc.tile_pool(name="sinvb", bufs=2) as sinvbp,
            tc.tile_pool(name="small", bufs=4) as smallp,
            tc.tile_pool(name="wslab", bufs=2) as wslabp,
            tc.tile_pool(name="wrhs", bufs=2) as wrhsp,
            tc.tile_pool(name="wfc2p", bufs=12) as wfc2p,
            tc.tile_pool(name="h2tmp", bufs=2) as h2tmpp,
            tc.tile_pool(name="xntok", bufs=2) as xntokp,
            tc.tile_pool(name="fepool", bufs=1) as fepool,
            tc.tile_pool(name="bigps", bufs=4, space="PSUM") as bigps,
        ):
            try:
                for _rep in range(repeat):
                    # ---- persistent tiles ----
                    x_sb = pp.tile([P, NT, C], F32, tag="x")            # 32KB
                    xnT = pp.tile([P, NCD, T], F8, tag="xnT")           # 8KB
                    xn2T = pp.tile([P, NCD, T], BF16, tag="xn2T")       # 16KB
                    v_sb = pp.tile([P, NT, (NH // 2) * 130], F8, tag="v")
                    yT = pp.tile([P, NCD, T], F8, tag="yT")             # 8KB
                    h2T = pp.tile([P, NFF, T // 2], F8, tag="h2T")      # 16KB
                    bproj_b = pp.tile([P, C], BF16, tag="bprojb")
                    bfc2_b = pp.tile([P, C], BF16, tag="bfc2b")
                    bqk_t = pp.tile([P, 16], F32, tag="bqkt")
                    mask_cols = pp.tile([P, NT], F32, tag="maskc")
                    mask_ab = pp.tile([P, NT], F32, tag="maskab")
                    bfc_t = pp.tile([P, NFF], F32, tag="bfct")

                    nc.sync.dma_start(bqk_t[:], bqk_t_d)
                    nc.sync.dma_start(mask_cols[:], mask_cols_d)
                    nc.sync.dma_start(mask_ab[:], mask_ab_d)
                    nc.sync.dma_start(bfc_t[:], bfc_t_d)
                    nc.sync.dma_start(bproj_b[0:1, :], bprojrow_d)
                    nc.gpsimd.partition_broadcast(bproj_b[:], bproj_b[0:1, :])
                    nc.sync.dma_start(bfc2_b[0:1, :], bfc2row_d)
                    nc.gpsimd.partition_broadcast(bfc2_b[:], bfc2_b[0:1, :])

                    # ---- prefetch V weights before the 4MB x stream so the
                    # V matmuls are not stuck behind it in the DMA queues ----
                    vslabs = []
                    for n in range(2):
                        slab = wrhsp.tile([P, 8, 512], F8, tag="wrhs",
                                          name=f"wv_{n}")
                        nc.sync.dma_start(slab[:], wv_v[n])
                        vslabs.append(slab)

                    # ---- load x (per-tile, so LN1 pipelines behind the DMA) ----
                    for i in range(NT):
                        nc.sync.dma_start(x_sb[:, i, :], x_v[:, i, :])

                    # ---- LayerNorm (token-major stats, DMA-transpose to
                    # feature-major dstT; fp8 dst goes via a bf16 scratch) ----
                    def layernorm_to_T(dstT, scratchT=None):
                        for i in range(NT):
                            xi = x_sb[:, i, :]
                            stats6 = smallp.tile([P, 2, 6], F32, tag="stats6")
                            nc.vector.bn_stats(stats6[:, 0, :], xi[:, 0:512])
                            nc.vector.bn_stats(stats6[:, 1, :], xi[:, 512:1024])
                            mv = smallp.tile([P, 2], F32, tag="mv")
                            nc.vector.bn_aggr(mv[:], stats6.rearrange("p a b -> p (a b)"))
                            rstd = smallp.tile([P, 1], F32, tag="rstd")
                            nc.vector.tensor_scalar_add(rstd[:], mv[:, 1:2], EPS)
                            nc.scalar.sqrt(rstd[:], rstd[:])
                            nc.vector.reciprocal(rstd[:], rstd[:])
                            negmr = smallp.tile([P, 1], F32, tag="negmr")
                            nc.vector.scalar_tensor_tensor(
                                negmr[:], mv[:, 0:1], -1.0, rstd[:],
                                op0=OP.mult, op1=OP.mult)
                            xn = xntokp.tile([P, C], BF16, tag="xntok")
                            nc.scalar.activation(xn[:], xi, AF.Identity,
                                                 bias=negmr[:], scale=rstd[:])
                            tgt = dstT if scratchT is None else scratchT
                            nc.sync.dma_start_transpose(
                                tgt[:, :, i * P:(i + 1) * P], xn[:])
                            if scratchT is not None:
                                nc.vector.tensor_copy(
                                    dstT[:, :, i * P:(i + 1) * P],
                                    scratchT[:, :, i * P:(i + 1) * P])

                    layernorm_to_T(xnT, scratchT=xn2T)

                    # ---- V = xn @ wv  (token-major, fp8 DoubleRow; head pairs
                    # packed [even64 | ones | ones | odd64] so both heads' AV
                    # matmuls at M=65 also yield the softmax sums) ----
                    if phases < 2:
                        raise _PhaseDone()
                    # per head pair: [even64 | ones | odd64 | ones2] (130 cols)
                    v_view = v_sb.rearrange("p i (pr e) -> p i pr e", e=130)
                    nc.gpsimd.memset(v_view[:, :, :, 64:65], 1.0)
                    nc.gpsimd.memset(v_view[:, :, :, 129:130], 1.0)
                    for n in range(2):
                        slab = vslabs[n]
                        for mt in range(NT):
                            ps = bigps.tile([P, 512], F32, tag="big", bufs=2)
                            for k2 in range(4):
                                nc.tensor.matmul(
                                    ps[:],
                                    xnT[:, 2 * k2:2 * k2 + 2, mt * P:(mt + 1) * P],
                                    slab[:, 2 * k2:2 * k2 + 2, :],
                                    start=(k2 == 0), stop=(k2 == 3),
                                    perf_mode=DR)
                            psv = ps.rearrange("p (pr two e) -> p pr two e",
                                               two=2, e=HD)
                            nc.vector.tensor_scalar(
                                v_view[:, mt, 4 * n:4 * (n + 1), 0:HD],
                                psv[:, :, 0, :], INV_WS, None, op0=OP.mult)
                            nc.vector.tensor_scalar(
                                v_view[:, mt, 4 * n:4 * (n + 1), 65:129],
                                psv[:, :, 1, :], INV_WS, None, op0=OP.mult)

                    if phases < 3:
                        raise _PhaseDone()
                    # ---- attention ----
                    # att^T[k, q] layout: keys on psum partitions.  Key-padding
                    # mask applied as the per-partition bias of the exp
                    # activation (exp(att - 3e4) == 0 for masked keys).
                    # zero the pad halves of the q tiles once (the buffers
                    # rotate but the pad halves are never written again)
                    for _zb in range(2):
                        qz_e = qpool.tile([P, T], BF16, tag="qe",
                                          name=f"qzini_e{_zb}")
                        nc.gpsimd.memset(qz_e[64:128, :], 0.0)
                        qz_o = qpool.tile([P, T], BF16, tag="qo",
                                          name=f"qzini_o{_zb}")
                        nc.gpsimd.memset(qz_o[0:64, :], 0.0)
                    for m in range(NH // 2):  # head pairs (2m, 2m+1)
                        q_e = qpool.tile([P, T], BF16, tag="qe", name=f"qe_{m}")
                        q_o = qpool.tile([P, T], BF16, tag="qo", name=f"qo_{m}")
                        k_sb = kpool.tile([P, T], BF16, tag="k", name=f"k_{m}")
                        for which, mm in ((0, m), (1, m + 8)):  # 0=q, 1=k
                            slab = wslabp.tile([P, 8, P], F8, tag="wslab",
                                               name=f"wqk_{m}_{which}")
                            nc.sync.dma_start(slab[:], wqk_v[mm])
                            for n in range(2):
                                ps = bigps.tile([P, 512], F32, tag="big", bufs=2)
                                for k2 in range(4):
                                    nc.tensor.matmul(
                                        ps[:], slab[:, 2 * k2:2 * k2 + 2, :],
                                        xnT[:, 2 * k2:2 * k2 + 2,
                                            n * 512:(n + 1) * 512],
                                        start=(k2 == 0), stop=(k2 == 3),
                                        perf_mode=DR)
                                if which == 1:
                                    nc.vector.tensor_scalar(
                                        k_sb[:, n * 512:(n + 1) * 512], ps[:],
                                        INV_WS, bqk_t[:, mm:mm + 1],
                                        op0=OP.mult, op1=OP.add)
                                else:
                                    nc.vector.tensor_scalar(
                                        q_e[0:64, n * 512:(n + 1) * 512],
                                        ps[0:64, :],
                                        INV_WS, bqk_t[0:64, mm:mm + 1],
                                        op0=OP.mult, op1=OP.add)
                                    nc.vector.tensor_scalar(
                                        q_o[64:128, n * 512:(n + 1) * 512],
                                        ps[64:128, :],
                                        INV_WS, bqk_t[64:128, mm:mm + 1],
                                        op0=OP.mult, op1=OP.add)

                        # --- interleaved QK / exp / AV so ScalarE (exp) never
                        # stalls: QK psums rotate through 2 slots (tag "big"),
                        # AV accumulators hold 2 dedicated slots (tag "av").
                        # Both heads' QK matmuls are issued back-to-back into
                        # disjoint PE row groups so they stream concurrently. ---
                        st_e = stpool.tile([P, NT, T], F8, tag="ste",
                                           name=f"ste_{m}")
                        st_o = stpool.tile([P, NT, T], F8, tag="sto",
                                           name=f"sto_{m}")
                        ps_y = bigps.tile([P, T], F32, tag="av", bufs=2,
                                          name=f"ye_{m}")
                        ps_y2 = bigps.tile([P, T], F32, tag="av", bufs=2,
                                           name=f"yo_{m}")
                        for t2 in range(4):
                            for kt in (2 * t2, 2 * t2 + 1):
                                ps_e = bigps.tile([P, T], F32, tag="big", bufs=2,
                                                  name=f"qke_{m}_{kt}")
                                ps_o = bigps.tile([P, T], F32, tag="big", bufs=2,
                                                  name=f"qko_{m}_{kt}")
                                for n in range(2):
                                    nc.tensor.matmul(
                                        ps_e[:, n * 512:(n + 1) * 512],
                                        k_sb[:, kt * P:(kt + 1) * P],
                                        q_e[:, n * 512:(n + 1) * 512],
                                        start=True, stop=True)
                                    nc.tensor.matmul(
                                        ps_o[:, n * 512:(n + 1) * 512],
                                        k_sb[:, kt * P:(kt + 1) * P],
                                        q_o[:, n * 512:(n + 1) * 512],
                                        start=True, stop=True)
                                nc.scalar.activation(st_e[:, kt, :], ps_e[:],
                                                     AF.Exp,
                                                     bias=mask_cols[:, kt:kt + 1])
                                if kt % 2 == 0:
                                    # Schraudolph fast-exp on DVE: magic-add
                                    # packs int(l*A/256+B/256) into the f32
                                    # mantissa; <<8 rebuilds exponent+mantissa.
                                    uf = fepool.tile([P, T], F32, tag="uf",
                                                     bufs=1)
                                    nc.vector.tensor_scalar(
                                        uf[:], ps_o[:], FE_A,
                                        mask_ab[:, kt:kt + 1],
                                        op0=OP.mult, op1=OP.add)
                                    t32 = fepool.tile([P, T], I32, tag="t32",
                                                      bufs=1)
                                    nc.vector.tensor_scalar(
                                        t32[:], uf[:].bitcast(I32), 8, None,
                                        op0=OP.logical_shift_left)
                                    nc.vector.tensor_copy(st_o[:, kt, :],
                                                          t32[:].bitcast(F32))
                                else:
                                    nc.scalar.activation(
                                        st_o[:, kt, :], ps_o[:], AF.Exp,
                                        bias=mask_cols[:, kt:kt + 1])
                            # AV for this key-tile pair (fp8 DoubleRow; the ones
                            # columns put row-sums at psum row 64 of each head)
                            for n in range(2):
                                nc.tensor.matmul(
                                    ps_y[0:65, n * 512:(n + 1) * 512],
                                    v_view[:, 2 * t2:2 * t2 + 2, m, 0:65],
                                    st_e[:, 2 * t2:2 * t2 + 2,
                                         n * 512:(n + 1) * 512],
                                    start=(t2 == 0), stop=(t2 == 3),
                                    perf_mode=DR, skip_group_check=True)
                                nc.tensor.matmul(
                                    ps_y2[0:65, n * 512:(n + 1) * 512],
                                    v_view[:, 2 * t2:2 * t2 + 2, m, 65:130],
                                    st_o[:, 2 * t2:2 * t2 + 2,
                                         n * 512:(n + 1) * 512],
                                    start=(t2 == 0), stop=(t2 == 3),
                                    perf_mode=DR, skip_group_check=True)
                        # yT = (y * 64) / rowsum, cast fp8.  Exact DVE
                        # reciprocal is ~6 cyc/elem; instead copy the sums row
                        # to SBUF and use the fast approximate reciprocal
                        # (PSUM input is broken on HW for the custom DVE op).
                        sumr_e = sinvbp.tile([1, T], F32, tag="sre", bufs=1)
                        nc.vector.tensor_copy(sumr_e[:], ps_y[64:65, :])
                        sumr_o = sinvbp.tile([1, T], F32, tag="sro", bufs=1)
                        nc.vector.tensor_copy(sumr_o[:], ps_y2[64:65, :])
                        sinv_be = sinvbp.tile([P, T], F32, tag="sbe", bufs=1)
                        nc.vector.reciprocal_approx_fast(sinv_be[0:1, :],
                                                         sumr_e[:])
                        nc.gpsimd.partition_broadcast(sinv_be[:], sinv_be[0:1, :])
                        sinv_bo = sinvbp.tile([P, T], F32, tag="sbo", bufs=1)
                        nc.vector.reciprocal_approx_fast(sinv_bo[0:1, :],
                                                         sumr_o[:])
                        nc.gpsimd.partition_broadcast(sinv_bo[:], sinv_bo[0:1, :])
                        nc.vector.scalar_tensor_tensor(
                            yT[0:64, m, :], ps_y[0:64, :], WS,
                            sinv_be[0:64, :], op0=OP.mult, op1=OP.mult)
                        # odd head: DVE stays partition-aligned at rows 0:64,
                        # then DMA shifts the rows to yT[64:128]
                        ytmp = sinvbp.tile([P, T], F8, tag="ytmp")
                        nc.vector.scalar_tensor_tensor(
                            ytmp[0:64, :], ps_y2[0:64, :], WS,
                            sinv_bo[0:64, :], op0=OP.mult, op1=OP.mult)
                        nc.sync.dma_start(yT[64:128, m, :], ytmp[0:64, :])

                    if phases < 4:
                        raise _PhaseDone()
                    # ---- residual prep: x += bproj_row ----
                    for i in range(NT):
                        nc.gpsimd.tensor_tensor(x_sb[:, i, :], x_sb[:, i, :],
                                                bproj_b[:], OP.add)

                    # ---- proj: x1 = x + y @ wproj (fp8 DoubleRow) ----
                    slabs = []
                    for n in range(2):
                        slab = wrhsp.tile([P, 8, 512], F8, tag="wrhs",
                                          name=f"wproj_{n}")
                        nc.sync.dma_start(slab[:], wproj_v[n])
                        slabs.append(slab)
                    for mt in range(NT):
                        for n in range(2):
                            ps = bigps.tile([P, 512], F32, tag="big", bufs=2)
                            for k2 in range(4):
                                nc.tensor.matmul(
                                    ps[:],
                                    yT[:, 2 * k2:2 * k2 + 2, mt * P:(mt + 1) * P],
                                    slabs[n][:, 2 * k2:2 * k2 + 2, :],
                                    start=(k2 == 0), stop=(k2 == 3),
                                    perf_mode=DR)
                            nc.vector.scalar_tensor_tensor(
                                x_sb[:, mt, n * 512:(n + 1) * 512], ps[:],
                                1.0 / (WS * WS),
                                x_sb[:, mt, n * 512:(n + 1) * 512],
                                op0=OP.mult, op1=OP.add)

                    if phases < 5:
                        raise _PhaseDone()
                    # ---- LN2 -> bf16 xn2T + fp8 xnT (for the fp8 FC1 part) ----
                    layernorm_to_T(xnT, scratchT=xn2T)

                    # ---- residual prep 2: x1 += bfc2_row ----
                    for i in range(NT):
                        nc.gpsimd.tensor_tensor(x_sb[:, i, :], x_sb[:, i, :],
                                                bfc2_b[:], OP.add)

                    if phases < 6:
                        raise _PhaseDone()
                    # ---- MLP (bf16) ----
                    for th in range(2):
                        tsl = slice(th * 512, (th + 1) * 512)
                        # FC1 (bf16): relu on ScalarE -> bf16 tmp, then
                        # GpSimd centers by -H2C and casts fp8 into h2T
                        for kk in range(NFF):
                            ps = bigps.tile([P, 512], F32, tag="big", bufs=2)
                            if kk < KK8:
                                slab = wslabp.tile([P, 8, P], F8, tag="wslab")
                                nc.sync.dma_start(slab[:], wfc8_v[kk])
                                for k2 in range(4):
                                    nc.tensor.matmul(
                                        ps[:], slab[:, 2 * k2:2 * k2 + 2, :],
                                        xnT[:, 2 * k2:2 * k2 + 2, tsl],
                                        start=(k2 == 0), stop=(k2 == 3),
                                        perf_mode=DR)
                                relu_scale = INV_WS
                            else:
                                slab = wslabp.tile([P, 8, P], BF16, tag="wslabf")
                                nc.sync.dma_start(slab[:], wfc_v[kk - KK8])
                                for ko in range(NCD):
                                    nc.tensor.matmul(ps[:], slab[:, ko, :],
                                                     xn2T[:, ko, tsl],
                                                     start=(ko == 0),
                                                     stop=(ko == NCD - 1))
                                relu_scale = 1.0
                            tmp = h2tmpp.tile([P, 512], BF16, tag="h2tmp")
                            nc.scalar.activation(tmp[:], ps[:], AF.Relu,
                                                 bias=bfc_t[:, kk:kk + 1],
                                                 scale=relu_scale)
                            nc.vector.tensor_scalar(h2T[:, kk, :], tmp[:],
                                                    -H2C, None, op0=OP.add)
                        # FC2: x2 = x1 + h2 @ wfc2
                        pw = [bigps.tile([P, T], F32, tag="big", bufs=2,
                                         name=f"fc2ps_{th}_{j}")
                              for j in range(2)] + \
                             [bigps.tile([P, T], F32, tag="av", bufs=2,
                                         name=f"fc2ps_{th}_{j + 2}")
                              for j in range(2)]
                        # accumulator (mt, ch) -> pw[2*ch + mt//2] half mt%2
                        pss = [[pw[2 * ch + mt // 2][:, (mt % 2) * 512:
                                                     (mt % 2) * 512 + 512]
                                for mt in range(4)] for ch in range(2)]
                        # W_fc2 streamed ONCE per th (fp8, kk-pair tiles for
                        # DoubleRow); j blocked by 4 so each accumulator gets
                        # 4 back-to-back matmuls
                        for jb in range(4):
                            rhss = []
                            for j4 in range(4):
                                j = jb * 4 + j4
                                pair = []
                                for ch in range(2):
                                    rhs = wfc2p.tile([P, 2, 512], F8, tag="wfc2",
                                                     name=f"wfc2_{th}_{j}_{ch}")
                                    nc.sync.dma_start(
                                        rhs.rearrange("p e c -> p (e c)"),
                                        wfc2_d[ch, :, j * 1024:(j + 1) * 1024])
                                    pair.append(rhs)
                                rhss.append(pair)
                            for ch in range(2):
                                for mt in range(4):
                                    for j4 in range(4):
                                        j = jb * 4 + j4
                                        nc.tensor.matmul(
                                            pss[ch][mt][:],
                                            h2T[:, 2 * j:2 * j + 2,
                                                mt * P:(mt + 1) * P],
                                            rhss[j4][ch][:],
                                            start=(j == 0), stop=(j == 15),
                                            perf_mode=DR,
                                            skip_group_check=True)
                        for ch in range(2):
                            for mt in range(4):
                                i = 4 * th + mt
                                nc.vector.scalar_tensor_tensor(
                                    x_sb[:, i, ch * 512:(ch + 1) * 512],
                                    pss[ch][mt][:], INV_WS,
                                    x_sb[:, i, ch * 512:(ch + 1) * 512],
                                    op0=OP.mult, op1=OP.add)
                        for mt in range(4):
                            i = 4 * th + mt
                            nc.sync.dma_start(out_v[:, i, :], x_sb[:, i, :])
            except _PhaseDone:
                pass

    nc.compile()
    return nc


class _PhaseDone(Exception):
    pass


_NC_CACHE = None


def _get_nc():
    global _NC_CACHE
    if _NC_CACHE is None:
        _NC_CACHE = _build_nc()
    return _NC_CACHE


def _run(inputs, trace=False, **kwargs):
    shared, per_core = _host_prep(**inputs)
    nc = _get_nc()
    in_maps = [{**shared, **pc} for pc in per_core]
    res = run_bass_kernel_spmd(nc, in_maps, core_ids=list(range(N_CORES)),
                               trace=trace, **kwargs)
    out = np.stack([res.results[i]["out"] for i in range(N_CORES)], axis=0)
    return out.astype(np.float32), res


def kernel(**inputs):
    return _run(inputs)[0]



# revision 76
# speedup vs baseline: 1.2233x; 1.2233x over previous
"""Trainium2 Bass kernel for a dense transformer block (pre-LN, masked attention).

Sharding: data-parallel over batch B=8 across the 8 NeuronCores — each core
processes one full batch element [T=1024, C=1024]; weights are replicated.
No collectives needed.

Per-core dataflow (single NeuronCore):
  - x loaded token-major [128, 8, 1024] (tokens on partitions).
  - LN1 stats token-major (bn_stats/bn_aggr), normalize on ScalarE,
    PE-transpose to feature-major xnT [C, T] stored fp8(e4m3).
  - QKV: fp8 DoubleRow matmuls (2 contraction rows per PE cell, ~1.5x) with
    weights pre-scaled by 64 on host so they sit in e4m3 normal range; the
    1/64 is folded into the PSUM eviction scale.  Q/K stored bf16 (the QK
    matmul has K=64 contraction, which fp8 cannot speed up).  V stored fp8
    per head pair as [even64 | ones | ones | odd64] (130 cols): the two ones
    columns make both heads' AV matmuls also emit the softmax row-sums.
  - QK: bf16, keys on psum partitions.  The two heads of a pair live on
    partition halves of q_sb/k_sb, and their matmuls are issued back-to-back
    so the PE runs them concurrently in disjoint row groups (K=64 each).
  - softmax: exp on ScalarE with the -30000 key-padding mask as bias
    (masked keys give exp == 0 exactly); st stored fp8 unnormalized.
  - AV: fp8 DoubleRow over key-tile pairs; even head -> psum rows 0:65
    (row 64 = sums), odd head -> rows 63:128 (row 63 = sums).  Eviction
    multiplies by broadcast 64/rowsum and casts yT to fp8.
  - proj: fp8 DoubleRow, eviction fused with residual add (scale 1/4096).
  - LN2 -> bf16 xn2T, MLP in bf16 (fp8 would breach the error budget):
    relu fc1 feature-major, fc2 token-major + residual, DMA out.
  - MLP: W_fc2 is streamed exactly once per T-half (both column halves per
    load) with 8 PSUM accumulators in 4 paired banks, kk blocked by 4 so each
    accumulator sees back-to-back matmuls (keeps the PE HAM clock warm).
    Weights are host-pre-tiled so every slab DMA is contiguous per partition.
"""

import os
import sys
import numpy as np
import ml_dtypes

for _p in ("/opt/trn_rl_repo", "/opt/pypackages"):
    if os.path.isdir(_p) and _p not in sys.path:
        sys.path.append(_p)

import concourse.bass as bass
import concourse.mybir as mybir
import concourse.tile as tile
from concourse import bacc
from concourse.bass_utils import run_bass_kernel_spmd

P = 128
B, T, C = 8, 1024, 1024
NH, HD = 16, 64
FF = 4 * C
EPS = 1e-5
NT = T // P      # 8 token tiles
NCD = C // P     # 8 feature tiles
NFF = FF // P    # 32 ff tiles
N_CORES = 8
MASK_VAL = -40.0
WS = 64.0        # fp8 weight pre-scale
INV_WS = 1.0 / WS
H2C = 0.5        # h2 centering offset for fp8 FC2 (zeros stay exact)
KK8 = 8          # leading FC1 kk-tiles in fp8 (rest bf16)
# Schraudolph fast-exp constants (DVE magic-number path), /256 scaled so the
# +2^23 magic add stays in range; the <<8 shift restores the exponent field.
FE_A = float(2**23 / np.log(2)) / 256.0
FE_B = ((127 << 23) - 486411) / 256.0 + float(2**23)

F32 = mybir.dt.float32
I32 = mybir.dt.int32
BF16 = mybir.dt.bfloat16
F8 = mybir.dt.float8e4
AF = mybir.ActivationFunctionType
OP = mybir.AluOpType
DR = mybir.MatmulPerfMode.DoubleRow

bf16 = ml_dtypes.bfloat16
f8 = ml_dtypes.float8_e4m3


# --------------------------------------------------------------------------
# host-side preparation: fold LN gains/biases into weights, build mask rows
# --------------------------------------------------------------------------
def _host_prep(x, seq_ls, ln1_g, ln1_b, w_qkv, b_qkv, w_proj, b_proj,
               ln2_g, ln2_b, w_fc, b_fc, w_fc2, b_fc2):
    f32 = np.float32
    ln1_g, ln1_b = ln1_g.astype(f32), ln1_b.astype(f32)
    w_qkv = w_qkv.astype(f32)

    wqkv_eff = ln1_g[:, None] * w_qkv                     # [C, 3C]
    bqkv_eff = ln1_b @ w_qkv + b_qkv.astype(f32)          # [3C]
    scale = np.float32(1.0 / np.sqrt(HD))
    wq = wqkv_eff[:, :C] * scale
    bq = bqkv_eff[:C] * scale
    wk = wqkv_eff[:, C:2 * C]
    bk = bqkv_eff[C:2 * C]
    wv = wqkv_eff[:, 2 * C:]
    bv = bqkv_eff[2 * C:]

    bproj_eff = bv @ w_proj.astype(f32) + b_proj.astype(f32)   # [C]

    wfc_eff = ln2_g.astype(f32)[:, None] * w_fc.astype(f32)    # [C, FF]
    bfc_eff = ln2_b.astype(f32) @ w_fc.astype(f32) + b_fc.astype(f32)

    wqk = np.concatenate([wq, wk], axis=1)                # [C, 2C]
    bqk_t = np.concatenate([bq, bk]).reshape(16, P).T.copy()   # [P, 16]
    bfc_t = bfc_eff.reshape(NFF, P).T.copy()              # [P, 32]

    def to_f8(w):
        return np.clip(w * WS, -240.0, 240.0).astype(f8)

    def tile_km(w, mw):
        # [K, M] -> [M//mw, P, K//P, mw] so a [P, K//P, mw] slab is contiguous
        K_, M_ = w.shape
        return np.ascontiguousarray(
            w.reshape(K_ // P, P, M_ // mw, mw).transpose(2, 1, 0, 3)
        ).reshape(M_ // mw, P, (K_ // P) * mw)

    shared = {
        "wqk": tile_km(to_f8(wqk), P),              # [16, P, 8*128] fp8
        "wv": tile_km(to_f8(wv), 512),              # [2, P, 8*512] fp8
        "bqk_t": bqk_t.astype(f32),
        "wproj": tile_km(to_f8(w_proj.astype(f32)), 512),  # [2, P, 8*512] fp8
        "bprojrow": bproj_eff.reshape(1, C).astype(bf16),
        "wfc8": tile_km(to_f8(wfc_eff[:, :KK8 * P]), P),   # [8, P, 8*128] fp8
        "wfc": tile_km(wfc_eff[:, KK8 * P:].astype(bf16), P),  # [24, P, 8*128]
        "bfc_t": bfc_t.astype(f32),
        # FC2 fp8: kk-pair-tiled rhs [ch, p, j, e, col] for DoubleRow; h2 is
        # centered by H2C on device, compensated via the bias (rank-1 fold).
        "wfc2": np.ascontiguousarray(
            to_f8(w_fc2.astype(f32)).reshape(16, 2, P, 2, 512)
            .transpose(3, 2, 0, 1, 4)).reshape(2, P, 16 * 1024),
        "bfc2row": (b_fc2.astype(f32) + H2C *
                    to_f8(w_fc2.astype(f32)).astype(f32).sum(0) / WS)
                   .reshape(1, C).astype(bf16),
    }
    per_core = []
    t_idx = np.arange(T)
    for b in range(B):
        mask = np.where(t_idx < int(seq_ls[b]), 0.0, MASK_VAL).astype(f32)
        per_core.append({
            "x": np.ascontiguousarray(x[b]).astype(f32),
            "mask_cols": mask.reshape(NT, P).T.copy(),   # [P, NT]
            "mask_ab": (FE_B + mask * FE_A).astype(f32)
                       .reshape(NT, P).T.copy(),          # [P, NT]
        })
    return shared, per_core


# --------------------------------------------------------------------------
# kernel build (single NeuronCore program, SPMD across 8 cores)
# --------------------------------------------------------------------------
def _build_nc(phases=99, repeat=1):
    nc = bacc.Bacc("TRN2", target_bir_lowering=False, debug=False,
                   num_devices=N_CORES)

    x_d = nc.dram_tensor("x", [T, C], F32, kind="ExternalInput").ap()
    mask_cols_d = nc.dram_tensor("mask_cols", [P, NT], F32,
                                 kind="ExternalInput").ap()
    wqk_d = nc.dram_tensor("wqk", [16, P, 8 * P], F8,
                           kind="ExternalInput").ap()
    wv_d = nc.dram_tensor("wv", [2, P, 8 * 512], F8,
                          kind="ExternalInput").ap()
    bqk_t_d = nc.dram_tensor("bqk_t", [P, 16], F32, kind="ExternalInput").ap()
    wproj_d = nc.dram_tensor("wproj", [2, P, 8 * 512], F8,
                             kind="ExternalInput").ap()
    bprojrow_d = nc.dram_tensor("bprojrow", [1, C], BF16, kind="ExternalInput").ap()
    wfc8_d = nc.dram_tensor("wfc8", [KK8, P, 8 * P], F8,
                            kind="ExternalInput").ap()
    wfc_d = nc.dram_tensor("wfc", [NFF - KK8, P, 8 * P], BF16,
                           kind="ExternalInput").ap()
    mask_ab_d = nc.dram_tensor("mask_ab", [P, NT], F32,
                               kind="ExternalInput").ap()
    bfc_t_d = nc.dram_tensor("bfc_t", [P, NFF], F32, kind="ExternalInput").ap()
    wfc2_d = nc.dram_tensor("wfc2", [2, P, 16 * 1024], F8,
                            kind="ExternalInput").ap()
    bfc2row_d = nc.dram_tensor("bfc2row", [1, C], BF16, kind="ExternalInput").ap()
    out_d = nc.dram_tensor("out", [T, C], F32, kind="ExternalOutput").ap()

    # DRAM access-pattern views
    x_v = x_d.rearrange("(i p) c -> p i c", p=P)          # [P, NT, C]
    out_v = out_d.rearrange("(i p) c -> p i c", p=P)
    wqk_v = wqk_d.rearrange("m p (k c) -> m p k c", k=NCD)    # [16,P,8,128]
    wv_v = wv_d.rearrange("m p (k c) -> m p k c", k=NCD)       # [2,P,8,512]
    wproj_v = wproj_d.rearrange("m p (k c) -> m p k c", k=NCD)
    wfc8_v = wfc8_d.rearrange("m p (k c) -> m p k c", k=NCD)   # [8,P,8,128]
    wfc_v = wfc_d.rearrange("m p (k c) -> m p k c", k=NCD)     # [24,P,8,128]

    with tile.TileContext(nc) as tc:
        with (
            tc.tile_pool(name="persist", bufs=1) as pp,
            tc.tile_pool(name="qpool", bufs=2) as qpool,
            tc.tile_pool(name="kpool", bufs=2) as kpool,
            tc.tile_pool(name="stpool", bufs=2) as stpool,
            tc.tile_pool(name="sinvb", bufs=2) as sinvbp,
            tc.tile_pool(name="small", bufs=4) as smallp,
            tc.tile_pool(name="wslab", bufs=2) as wslabp,
            tc.tile_pool(name="wrhs", bufs=2) as wrhsp,
            tc.tile_pool(name="wfc2p", bufs=12) as wfc2p,
            tc.tile_pool(name="h2tmp", bufs=2) as h2tmpp,
            tc.tile_pool(name="xntok", bufs=2) as xntokp,
            tc.tile_pool(name="fepool", bufs=1) as fepool,
            tc.tile_pool(name="bigps", bufs=4, space="PSUM") as bigps,
        ):
            try:
                for _rep in range(repeat):
                    # ---- persistent tiles ----
                    x_sb = pp.tile([P, NT, C], F32, tag="x")            # 32KB
                    xnT = pp.tile([P, NCD, T], F8, tag="xnT")           # 8KB
                    xn2T = pp.tile([P, NCD, T], BF16, tag="xn2T")       # 16KB
                    v_sb = pp.tile([P, NT, (NH // 2) * 130], F8, tag="v")
                    yT = pp.tile([P, NCD, T], F8, tag="yT")             # 8KB
                    h2T = pp.tile([P, NFF, T // 2], F8, tag="h2T")      # 16KB
                    bproj_b = pp.tile([P, C], BF16, tag="bprojb")
                    bfc2_b = pp.tile([P, C], BF16, tag="bfc2b")
                    bqk_t = pp.tile([P, 16], F32, tag="bqkt")
                    mask_cols = pp.tile([P, NT], F32, tag="maskc")
                    mask_ab = pp.tile([P, NT], F32, tag="maskab")
                    bfc_t = pp.tile([P, NFF], F32, tag="bfct")

                    # ---- load x FIRST (LN1 gates everything; consts and wv
                    # ride the Act HWDGE queue so x owns SP from cycle 0) ----
                    for i in range(NT):
                        nc.sync.dma_start(x_sb[:, i, :], x_v[:, i, :])

                    nc.scalar.dma_start(bqk_t[:], bqk_t_d)
                    nc.scalar.dma_start(mask_cols[:], mask_cols_d)
                    nc.scalar.dma_start(mask_ab[:], mask_ab_d)
                    nc.scalar.dma_start(bfc_t[:], bfc_t_d)
                    nc.scalar.dma_start(bproj_b[0:1, :], bprojrow_d)
                    nc.gpsimd.partition_broadcast(bproj_b[:], bproj_b[0:1, :])
                    nc.scalar.dma_start(bfc2_b[0:1, :], bfc2row_d)
                    nc.gpsimd.partition_broadcast(bfc2_b[:], bfc2_b[0:1, :])

                    # ---- V weights (needed ~15us in, after LN1 tile 0) ----
                    vslabs = []
                    for n in range(2):
                        slab = wrhsp.tile([P, 8, 512], F8, tag="wrhs",
                                          name=f"wv_{n}")
                        nc.sync.dma_start(slab[:], wv_v[n])
                        vslabs.append(slab)

                    # ---- LayerNorm (token-major stats, DMA-transpose to
                    # feature-major dstT; fp8 dst goes via a bf16 scratch) ----
                    def layernorm_to_T(dstT, scratchT=None, resid_bias=None):
                        for i in range(NT):
                            xi = x_sb[:, i, :]
                            stats6 = smallp.tile([P, 2, 6], F32, tag="stats6")
                            nc.vector.bn_stats(stats6[:, 0, :], xi[:, 0:512])
                            nc.vector.bn_stats(stats6[:, 1, :], xi[:, 512:1024])
                            mv = smallp.tile([P, 2], F32, tag="mv")
                            nc.vector.bn_aggr(mv[:], stats6.rearrange("p a b -> p (a b)"))
                            rstd = smallp.tile([P, 1], F32, tag="rstd")
                            nc.vector.tensor_scalar_add(rstd[:], mv[:, 1:2], EPS)
                            nc.scalar.sqrt(rstd[:], rstd[:])
                            nc.vector.reciprocal(rstd[:], rstd[:])
                            negmr = smallp.tile([P, 1], F32, tag="negmr")
                            nc.vector.scalar_tensor_tensor(
                                negmr[:], mv[:, 0:1], -1.0, rstd[:],
                                op0=OP.mult, op1=OP.mult)
                            xn = xntokp.tile([P, C], BF16, tag="xntok")
                            nc.scalar.activation(xn[:], xi, AF.Identity,
                                                 bias=negmr[:], scale=rstd[:])
                            tgt = dstT if scratchT is None else scratchT
                            # transpose on the Act HWDGE queue so it is not
                            # stuck behind the x / weight streams on SP
                            nc.scalar.dma_start_transpose(
                                tgt[:, :, i * P:(i + 1) * P], xn[:])
                            if scratchT is not None:
                                nc.vector.tensor_copy(
                                    dstT[:, :, i * P:(i + 1) * P],
                                    scratchT[:, :, i * P:(i + 1) * P])
                            if resid_bias is not None:
                                # residual bias row lands right after this
                                # tile's last LN read -> overlaps the next
                                # phase instead of stalling after it
                                nc.gpsimd.tensor_tensor(
                                    x_sb[:, i, :], x_sb[:, i, :],
                                    resid_bias[:], OP.add)

                    layernorm_to_T(xnT, scratchT=xn2T, resid_bias=bproj_b)

                    # ---- V = xn @ wv  (token-major, fp8 DoubleRow; head pairs
                    # packed [even64 | ones | ones | odd64] so both heads' AV
                    # matmuls at M=65 also yield the softmax sums) ----
                    if phases < 2:
                        raise _PhaseDone()
                    # per head pair: [even64 | ones | odd64 | ones2] (130 cols)
                    v_view = v_sb.rearrange("p i (pr e) -> p i pr e", e=130)
                    nc.gpsimd.memset(v_view[:, :, :, 64:65], 1.0)
                    nc.gpsimd.memset(v_view[:, :, :, 129:130], 1.0)
                    for n in range(2):
                        slab = vslabs[n]
                        for mt in range(NT):
                            ps = bigps.tile([P, 512], F32, tag="big", bufs=2)
                            for k2 in range(4):
                                nc.tensor.matmul(
                                    ps[:],
                                    xnT[:, 2 * k2:2 * k2 + 2, mt * P:(mt + 1) * P],
                                    slab[:, 2 * k2:2 * k2 + 2, :],
                                    start=(k2 == 0), stop=(k2 == 3),
                                    perf_mode=DR)
                            psv = ps.rearrange("p (pr two e) -> p pr two e",
                                               two=2, e=HD)
                            nc.vector.tensor_scalar(
                                v_view[:, mt, 4 * n:4 * (n + 1), 0:HD],
                                psv[:, :, 0, :], INV_WS, None, op0=OP.mult)
                            nc.vector.tensor_scalar(
                                v_view[:, mt, 4 * n:4 * (n + 1), 65:129],
                                psv[:, :, 1, :], INV_WS, None, op0=OP.mult)

                    # prefetch wproj now (reuses the wv slots once the V
                    # matmuls drain) so proj never waits on its weights
                    projslabs = []
                    for n in range(2):
                        slab = wrhsp.tile([P, 8, 512], F8, tag="wrhs",
                                          name=f"wproj_{n}")
                        nc.sync.dma_start(slab[:], wproj_v[n])
                        projslabs.append(slab)

                    if phases < 3:
                        raise _PhaseDone()
                    # ---- attention ----
                    # att^T[k, q] layout: keys on psum partitions.  Key-padding
                    # mask applied as the per-partition bias of the exp
                    # activation (exp(att - 3e4) == 0 for masked keys).
                    # zero the pad halves of the q tiles once (the buffers
                    # rotate but the pad halves are never written again)
                    for _zb in range(2):
                        qz_e = qpool.tile([P, T], BF16, tag="qe",
                                          name=f"qzini_e{_zb}")
                        nc.gpsimd.memset(qz_e[64:128, :], 0.0)
                        qz_o = qpool.tile([P, T], BF16, tag="qo",
                                          name=f"qzini_o{_zb}")
                        nc.gpsimd.memset(qz_o[0:64, :], 0.0)
                    pending_tail = None
                    for m in range(NH // 2):  # head pairs (2m, 2m+1)
                        q_e = qpool.tile([P, T], BF16, tag="qe", name=f"qe_{m}")
                        q_o = qpool.tile([P, T], BF16, tag="qo", name=f"qo_{m}")
                        k_sb = kpool.tile([P, T], BF16, tag="k", name=f"k_{m}")
                        for which, mm in ((0, m), (1, m + 8)):  # 0=q, 1=k
                            slab = wslabp.tile([P, 8, P], F8, tag="wslab",
                                               name=f"wqk_{m}_{which}")
                            # Act HWDGE queue: the SP queue's data-dependent
                            # yT-shift DMA would otherwise stall this prefetch
                            # (the per-pair PE bubble in the profile)
                            nc.scalar.dma_start(slab[:], wqk_v[mm])
                            for n in range(2):
                                ps = bigps.tile([P, 512], F32, tag="big", bufs=2)
                                for k2 in range(4):
                                    nc.tensor.matmul(
                                        ps[:], slab[:, 2 * k2:2 * k2 + 2, :],
                                        xnT[:, 2 * k2:2 * k2 + 2,
                                            n * 512:(n + 1) * 512],
                                        start=(k2 == 0), stop=(k2 == 3),
                                        perf_mode=DR)
                                if which == 1:
                                    nc.vector.tensor_scalar(
                                        k_sb[:, n * 512:(n + 1) * 512], ps[:],
                                        INV_WS, bqk_t[:, mm:mm + 1],
                                        op0=OP.mult, op1=OP.add)
                                else:
                                    nc.vector.tensor_scalar(
                                        q_e[0:64, n * 512:(n + 1) * 512],
                                        ps[0:64, :],
                                        INV_WS, bqk_t[0:64, mm:mm + 1],
                                        op0=OP.mult, op1=OP.add)
                                    nc.vector.tensor_scalar(
                                        q_o[64:128, n * 512:(n + 1) * 512],
                                        ps[64:128, :],
                                        INV_WS, bqk_t[64:128, mm:mm + 1],
                                        op0=OP.mult, op1=OP.add)

                        # previous pair's deferred normalize: emitted AFTER
                        # this pair's q/k evictions so the DVE queue serves
                        # those first (kills the per-pair PE stall)
                        if pending_tail is not None:
                            pending_tail()
                            pending_tail = None

                        # --- interleaved QK / exp / AV so ScalarE (exp) never
                        # stalls: QK psums rotate through 2 slots (tag "big"),
                        # AV accumulators hold 2 dedicated slots (tag "av").
                        # Both heads' QK matmuls are issued back-to-back into
                        # disjoint PE row groups so they stream concurrently. ---
                        st_e = stpool.tile([P, NT, T], F8, tag="ste",
                                           name=f"ste_{m}")
                        st_o = stpool.tile([P, NT, T], F8, tag="sto",
                                           name=f"sto_{m}")
                        ps_y = bigps.tile([P, T], F32, tag="av", bufs=2,
                                          name=f"ye_{m}")
                        ps_y2 = bigps.tile([P, T], F32, tag="av", bufs=2,
                                           name=f"yo_{m}")
                        for t2 in range(4):
                            for kt in (2 * t2, 2 * t2 + 1):
                                ps_e = bigps.tile([P, T], F32, tag="big", bufs=2,
                                                  name=f"qke_{m}_{kt}")
                                ps_o = bigps.tile([P, T], F32, tag="big", bufs=2,
                                                  name=f"qko_{m}_{kt}")
                                for n in range(2):
                                    nc.tensor.matmul(
                                        ps_e[:, n * 512:(n + 1) * 512],
                                        k_sb[:, kt * P:(kt + 1) * P],
                                        q_e[:, n * 512:(n + 1) * 512],
                                        start=True, stop=True)
                                    nc.tensor.matmul(
                                        ps_o[:, n * 512:(n + 1) * 512],
                                        k_sb[:, kt * P:(kt + 1) * P],
                                        q_o[:, n * 512:(n + 1) * 512],
                                        start=True, stop=True)
                                nc.scalar.activation(st_e[:, kt, :], ps_e[:],
                                                     AF.Exp,
                                                     bias=mask_cols[:, kt:kt + 1])
                                if kt % 2 == 0:
                                    # Schraudolph fast-exp on DVE: magic-add
                                    # packs int(l*A/256+B/256) into the f32
                                    # mantissa; <<8 rebuilds exponent+mantissa.
                                    uf = fepool.tile([P, T], F32, tag="uf",
                                                     bufs=1)
                                    nc.vector.tensor_scalar(
                                        uf[:], ps_o[:], FE_A,
                                        mask_ab[:, kt:kt + 1],
                                        op0=OP.mult, op1=OP.add)
                                    t32 = fepool.tile([P, T], I32, tag="t32",
                                                      bufs=1)
                                    nc.vector.tensor_scalar(
                                        t32[:], uf[:].bitcast(I32), 8, None,
                                        op0=OP.logical_shift_left)
                                    nc.vector.tensor_copy(st_o[:, kt, :],
                                                          t32[:].bitcast(F32))
                                else:
                                    nc.scalar.activation(
                                        st_o[:, kt, :], ps_o[:], AF.Exp,
                                        bias=mask_cols[:, kt:kt + 1])
                            # AV for this key-tile pair (fp8 DoubleRow; the ones
                            # columns put row-sums at psum row 64 of each head)
                            for n in range(2):
                                nc.tensor.matmul(
                                    ps_y[0:65, n * 512:(n + 1) * 512],
                                    v_view[:, 2 * t2:2 * t2 + 2, m, 0:65],
                                    st_e[:, 2 * t2:2 * t2 + 2,
                                         n * 512:(n + 1) * 512],
                                    start=(t2 == 0), stop=(t2 == 3),
                                    perf_mode=DR, skip_group_check=True)
                                nc.tensor.matmul(
                                    ps_y2[0:65, n * 512:(n + 1) * 512],
                                    v_view[:, 2 * t2:2 * t2 + 2, m, 65:130],
                                    st_o[:, 2 * t2:2 * t2 + 2,
                                         n * 512:(n + 1) * 512],
                                    start=(t2 == 0), stop=(t2 == 3),
                                    perf_mode=DR, skip_group_check=True)
                        # yT = (y * 64) / rowsum, cast fp8.  The sums-row
                        # copies run NOW (cheap), but the recip/broadcast/
                        # normalize chain is DEFERRED until after the next
                        # pair's q/k evictions so it never delays them in
                        # the in-order DVE queue (the 8.7us/pair PE stall).
                        sumr_e = sinvbp.tile([1, T], F32, tag="sre", bufs=1)
                        nc.vector.tensor_copy(sumr_e[:], ps_y[64:65, :])
                        sumr_o = sinvbp.tile([1, T], F32, tag="sro", bufs=1)
                        nc.vector.tensor_copy(sumr_o[:], ps_y2[64:65, :])

                        def make_tail(m, ps_y, ps_y2, sumr_e, sumr_o):
                            def tail():
                                sinv_be = sinvbp.tile([P, T], F32, tag="sbe",
                                                      bufs=1)
                                nc.vector.reciprocal_approx_fast(
                                    sinv_be[0:1, :], sumr_e[:])
                                nc.gpsimd.partition_broadcast(
                                    sinv_be[:], sinv_be[0:1, :])
                                sinv_bo = sinvbp.tile([P, T], F32, tag="sbo",
                                                      bufs=1)
                                nc.vector.reciprocal_approx_fast(
                                    sinv_bo[0:1, :], sumr_o[:])
                                nc.gpsimd.partition_broadcast(
                                    sinv_bo[:], sinv_bo[0:1, :])
                                nc.vector.scalar_tensor_tensor(
                                    yT[0:64, m, :], ps_y[0:64, :], WS,
                                    sinv_be[0:64, :], op0=OP.mult,
                                    op1=OP.mult)
                                # odd head: DVE stays partition-aligned at
                                # rows 0:64, then DMA shifts to yT[64:128]
                                ytmp = sinvbp.tile([P, T], F8, tag="ytmp")
                                nc.vector.scalar_tensor_tensor(
                                    ytmp[0:64, :], ps_y2[0:64, :], WS,
                                    sinv_bo[0:64, :], op0=OP.mult,
                                    op1=OP.mult)
                                nc.sync.dma_start(yT[64:128, m, :],
                                                  ytmp[0:64, :])
                            return tail

                        pending_tail = make_tail(m, ps_y, ps_y2,
                                                 sumr_e, sumr_o)

                    pending_tail()

                    if phases < 4:
                        raise _PhaseDone()
                    # (x += bproj_row already applied inside the LN1 loop,
                    # overlapped with attention)

                    # ---- proj: x1 = x + y @ wproj (fp8 DoubleRow;
                    # slabs were prefetched during attention) ----
                    slabs = projslabs
                    for mt in range(NT):
                        for n in range(2):
                            ps = bigps.tile([P, 512], F32, tag="big", bufs=2)
                            for k2 in range(4):
                                nc.tensor.matmul(
                                    ps[:],
                                    yT[:, 2 * k2:2 * k2 + 2, mt * P:(mt + 1) * P],
                                    slabs[n][:, 2 * k2:2 * k2 + 2, :],
                                    start=(k2 == 0), stop=(k2 == 3),
                                    perf_mode=DR)
                            nc.vector.scalar_tensor_tensor(
                                x_sb[:, mt, n * 512:(n + 1) * 512], ps[:],
                                1.0 / (WS * WS),
                                x_sb[:, mt, n * 512:(n + 1) * 512],
                                op0=OP.mult, op1=OP.add)

                    if phases < 5:
                        raise _PhaseDone()
                    # ---- LN2 -> bf16 xn2T + fp8 xnT (for the fp8 FC1 part);
                    # x1 += bfc2_row folded into the loop, overlapping FC1 ----
                    layernorm_to_T(xnT, scratchT=xn2T, resid_bias=bfc2_b)

                    if phases < 6:
                        raise _PhaseDone()
                    # ---- MLP (bf16) ----
                    for th in range(2):
                        tsl = slice(th * 512, (th + 1) * 512)
                        # FC1 (bf16): relu on ScalarE -> bf16 tmp, then
                        # GpSimd centers by -H2C and casts fp8 into h2T
                        for kk in range(NFF):
                            ps = bigps.tile([P, 512], F32, tag="big", bufs=2)
                            if kk < KK8:
                                slab = wslabp.tile([P, 8, P], F8, tag="wslab")
                                nc.sync.dma_start(slab[:], wfc8_v[kk])
                                for k2 in range(4):
                                    nc.tensor.matmul(
                                        ps[:], slab[:, 2 * k2:2 * k2 + 2, :],
                                        xnT[:, 2 * k2:2 * k2 + 2, tsl],
                                        start=(k2 == 0), stop=(k2 == 3),
                                        perf_mode=DR)
                                relu_scale = INV_WS
                            else:
                                slab = wslabp.tile([P, 8, P], BF16, tag="wslabf")
                                nc.sync.dma_start(slab[:], wfc_v[kk - KK8])
                                for ko in range(NCD):
                                    nc.tensor.matmul(ps[:], slab[:, ko, :],
                                                     xn2T[:, ko, tsl],
                                                     start=(ko == 0),
                                                     stop=(ko == NCD - 1))
                                relu_scale = 1.0
                            tmp = h2tmpp.tile([P, 512], BF16, tag="h2tmp")
                            nc.scalar.activation(tmp[:], ps[:], AF.Relu,
                                                 bias=bfc_t[:, kk:kk + 1],
                                                 scale=relu_scale)
                            nc.vector.tensor_scalar(h2T[:, kk, :], tmp[:],
                                                    -H2C, None, op0=OP.add)
                        # FC2: x2 = x1 + h2 @ wfc2
                        pw = [bigps.tile([P, T], F32, tag="big", bufs=2,
                                         name=f"fc2ps_{th}_{j}")
                              for j in range(2)] + \
                             [bigps.tile([P, T], F32, tag="av", bufs=2,
                                         name=f"fc2ps_{th}_{j + 2}")
                              for j in range(2)]
                        # accumulator (mt, ch) -> pw[2*ch + mt//2] half mt%2
                        pss = [[pw[2 * ch + mt // 2][:, (mt % 2) * 512:
                                                     (mt % 2) * 512 + 512]
                                for mt in range(4)] for ch in range(2)]
                        # W_fc2 streamed ONCE per th (fp8, kk-pair tiles for
                        # DoubleRow); j blocked by 4 so each accumulator gets
                        # 4 back-to-back matmuls
                        for jb in range(4):
                            rhss = []
                            for j4 in range(4):
                                j = jb * 4 + j4
                                pair = []
                                for ch in range(2):
                                    rhs = wfc2p.tile([P, 2, 512], F8, tag="wfc2",
                                                     name=f"wfc2_{th}_{j}_{ch}")
                                    # Act HWDGE queue: keeps the th=1 rhs
                                    # stream from queuing behind the th=0
                                    # output stores on SP
                                    nc.scalar.dma_start(
                                        rhs.rearrange("p e c -> p (e c)"),
                                        wfc2_d[ch, :, j * 1024:(j + 1) * 1024])
                                    pair.append(rhs)
                                rhss.append(pair)
                            for ch in range(2):
                                for mt in range(4):
                                    for j4 in range(4):
                                        j = jb * 4 + j4
                                        nc.tensor.matmul(
                                            pss[ch][mt][:],
                                            h2T[:, 2 * j:2 * j + 2,
                                                mt * P:(mt + 1) * P],
                                            rhss[j4][ch][:],
                                            start=(j == 0), stop=(j == 15),
                                            perf_mode=DR,
                                            skip_group_check=True)
                        for ch in range(2):
                            for mt in range(4):
                                i = 4 * th + mt
                                nc.vector.scalar_tensor_tensor(
                                    x_sb[:, i, ch * 512:(ch + 1) * 512],
                                    pss[ch][mt][:], INV_WS,
                                    x_sb[:, i, ch * 512:(ch + 1) * 512],
                                    op0=OP.mult, op1=OP.add)
                        for mt in range(4):
                            i = 4 * th + mt
                            nc.sync.dma_start(out_v[:, i, :], x_sb[:, i, :])
            except _PhaseDone:
                pass

    nc.compile()
    return nc


class _PhaseDone(Exception):
    pass


_NC_CACHE = None


def _get_nc():
    global _NC_CACHE
    if _NC_CACHE is None:
        _NC_CACHE = _build_nc()
    return _NC_CACHE


def _run(inputs, trace=False, **kwargs):
    shared, per_core = _host_prep(**inputs)
    nc = _get_nc()
    in_maps = [{**shared, **pc} for pc in per_core]
    res = run_bass_kernel_spmd(nc, in_maps, core_ids=list(range(N_CORES)),
                               trace=trace, **kwargs)
    out = np.stack([res.results[i]["out"] for i in range(N_CORES)], axis=0)
    return out.astype(np.float32), res


def kernel(**inputs):
    return _run(inputs)[0]

